# revision 3
# baseline (speedup 1.0000x reference)
"""MoA (mixture-of-adapters) dense-routing kernel for 8 Trainium2 NeuronCores.

Data-parallel over batch: core i computes batch row i entirely locally
(weights replicated), no collectives.

Math per token t (D=1024, E=8, H=128):
    probs = softmax(x @ Wr + br)                  [E]
    down_e = gelu(x @ Wd[e] + bd[e])              [H]
    out    = sum_e probs[e] * (down_e @ Wu[e] + bu[e])
Restructured as in the fp32r baseline:
    w_e    = exp(logit_e)          (unnormalized; exp via tanh)
    sc_e   = gelu(down_e) * w_e
    out    = (sum_e sc_e @ Wu[e] + sum_e w_e * bu[e]) * (1/sum_e w_e)

fp8 acceleration: the big matmuls run in float8e4 (e4m3) with the DoubleRow
perf mode: two K-tiles of 128 contracted per pass at 0.5 cycles/output-row,
4x the fp32r rate.  Precision is recovered with first-order residual
compensation: each operand X is stored as fp8(X) plus fp8(X - fp8(X)) at the
SAME scale (fp8 is floating point, the residual just uses smaller
exponents), and extra DoubleRow passes accumulate the cross terms into the
same PSUM group:
    x @ Wd ~= xq@wdq + xlo@wdq + xq@wdlo        (second order dropped)
The only uncompensated source is the on-device fp8 rounding of sc_e (the DVE
op that multiplies gelu acts by the router weights must emit fp8 for the up
matmul).  Simulated end-to-end rel err: 1.63e-2 vs the 2e-2 gate.

Engine placement per 512-token block:
    PE:   router 8 DR + down 8x12 DR + up 8x(1 f32r bias + 8 DR) + sums
    ACT:  8 gelu + 8 final Copy-with-scale (rinv), one act table set
    DVE:  router exp pipeline + 8 sc multiplies (fp8 out)
    Pool: 8 partition_broadcasts of the router weight rows (pbc)
Output is written bf16 and widened on host.
"""

import sys

sys.path.insert(0, "/opt/trn_rl_repo")

import ml_dtypes
import numpy as np
import concourse.bacc as bacc
import concourse.mybir as mybir
import concourse.tile as tile
from concourse.bass_utils import run_bass_kernel_spmd

F32 = mybir.dt.float32
F32R = mybir.dt.float32r
F8 = mybir.dt.float8e4
BF16 = mybir.dt.bfloat16
AF = mybir.ActivationFunctionType
ALU = mybir.AluOpType
DR = mybir.MatmulPerfMode.DoubleRow
E4M3 = ml_dtypes.float8_e4m3

B, T, D = 8, 2048, 1024
E, H = 8, 128
N_CORES = 8
TOK = T
BLK = 512
NBLK = TOK // BLK            # 4
NSUB = BLK // 128            # 4
NCH = D // 128               # 8 contraction chunks
NCP = NCH // 2               # 4 chunk pairs (DoubleRow)
NDC = D // 512               # 2 output D chunks

SX = 32.0                    # x pre-scale (max |x*SX| ~ 174 < 240)
SWD = 1024.0                 # Wd pre-scale
SWR = 1024.0                 # Wr pre-scale
SWU = 1024.0                 # Wu pre-scale

USE_POOL_BCAST = True


def _to_f32r(a: np.ndarray) -> np.ndarray:
    b = np.ascontiguousarray(a, dtype=np.float32).view(np.uint32).copy()
    low = b & np.uint32(0xFFF)
    b &= np.uint32(0xFFFFF000)
    lsb = (b >> np.uint32(12)) & np.uint32(1)
    round_up = (low > 0x800) | ((low == 0x800) & (lsb == 1))
    b += round_up.astype(np.uint32) << np.uint32(12)
    return b.view(np.float32)


def _q8(a: np.ndarray, scale: float):
    """Return (fp8 main, fp8 residual) of a*scale, both e4m3 at that scale."""
    hi = (np.asarray(a, np.float32) * scale).astype(E4M3)
    lo = (np.asarray(a, np.float32) * scale - hi.astype(np.float32)).astype(E4M3)
    return hi, lo


# f32 const blob layout (columns; F32R dram tensor, some slices bitcast F32)
CF_BU = 0          # [0:8, 0:1024]   bu * SWU (f32r)
CF_SEL = 1024      # [0:8, 1024:2048] sel rows (1.0 at row e cols e*128..)  (matmul fallback)
CF_BRH = 2048      # [0:8, 2048]     0.5 * br (f32)
CF_ONES = 2050     # [0:8, 2050:2052] SWU (sum-matmul column)
CF_BDT = 2052      # [:, 2052:2060]  bd.T (f32, gelu bias per expert)
CF_W = 2060


def build_nc(loop_n=1):
    nc = bacc.Bacc("TRN2", target_bir_lowering=False, debug=False,
                   num_devices=N_CORES)

    xq = nc.dram_tensor("xq", [128, NBLK * NCH * BLK], F8, kind="ExternalInput")
    xlo = nc.dram_tensor("xlo", [128, NBLK * NCH * BLK], F8, kind="ExternalInput")
    wd = nc.dram_tensor("wd", [128, E * NCH * H], F8, kind="ExternalInput")
    wdlo = nc.dram_tensor("wdlo", [128, E * NCH * H], F8, kind="ExternalInput")
    wu = nc.dram_tensor("wu", [128, 2 * E * BLK], F8, kind="ExternalInput")
    wulo = nc.dram_tensor("wulo", [128, 2 * E * BLK], F8, kind="ExternalInput")
    # router stationary: [128, (cp,i) = 8, 16] with columns [wr_hi_e | wr_lo_e]
    c8 = nc.dram_tensor("c8", [128, NCH * 16], F8, kind="ExternalInput")
    cf = nc.dram_tensor("cf", [128, CF_W], F32R, kind="ExternalInput")
    out = nc.dram_tensor("out", [TOK, D], BF16, kind="ExternalOutput")

    with tile.TileContext(nc) as tc:
        with tc.tile_pool(name="const", bufs=1) as cpool, \
             tc.tile_pool(name="xtp", bufs=3) as xtp, \
             tc.tile_pool(name="rt", bufs=2) as rt, \
             tc.tile_pool(name="work", bufs=2) as work, \
             tc.tile_pool(name="actp", bufs=4) as actp, \
             tc.tile_pool(name="scp", bufs=2) as scp, \
             tc.tile_pool(name="outp", bufs=3) as outp, \
             tc.tile_pool(name="ps_r", bufs=2, space="PSUM") as ps_r, \
             tc.tile_pool(name="ps_d", bufs=3, space="PSUM") as ps_d, \
             tc.tile_pool(name="ps_u", bufs=3, space="PSUM") as ps_u:

            wd_t = cpool.tile([128, E * NCH, H], F8)
            wdlo_t = cpool.tile([128, E * NCH, H], F8)
            wu_t = cpool.tile([128, 4 * NDC * 2, BLK], F8)
            wulo_t = cpool.tile([128, 4 * NDC * 2, BLK], F8)
            c8_t = cpool.tile([128, NCH, 16], F8)
            cf_t = cpool.tile([128, CF_W], F32R)

            bu_t = cf_t[0:E, CF_BU:CF_BU + D]
            sel_t = [cf_t[0:E, CF_SEL + e * 128:CF_SEL + (e + 1) * 128]
                     for e in range(E)]
            brh_t = cf_t[0:E, CF_BRH:CF_BRH + 1].bitcast(F32)
            ones_t = cf_t[0:E, CF_ONES:CF_ONES + 2]
            bdt_t = cf_t[:, CF_BDT:CF_BDT + 8].bitcast(F32)

            nc.sync.dma_start(c8_t[:], c8[:])
            nc.sync.dma_start(cf_t[:], cf[:])

            def load_weights(chunked=False):
                for e in range(E):
                    nc.sync.dma_start(
                        wd_t[:, e * NCH:(e + 1) * NCH, :],
                        wd[:, e * NCH * H:(e + 1) * NCH * H])
                    nc.sync.dma_start(
                        wdlo_t[:, e * NCH:(e + 1) * NCH, :],
                        wdlo[:, e * NCH * H:(e + 1) * NCH * H])
                nc.sync.dma_start(wu_t[:], wu[:])
                nc.sync.dma_start(wulo_t[:], wulo[:])

            def emit_body(weights_after_first_xt=False):
                for blk in range(NBLK):
                    xq_t = xtp.tile([128, NCH, BLK], F8, tag="xq")
                    xlo_t = xtp.tile([128, NCH, BLK], F8, tag="xlo")
                    xbase = blk * NCH * BLK
                    for cp in range(NCP):
                        nc.sync.dma_start(
                            xq_t[:, 2 * cp:2 * cp + 2, :],
                            xq[:, xbase + cp * 2 * BLK:xbase + (cp + 1) * 2 * BLK])
                    for cp in range(NCP):
                        nc.sync.dma_start(
                            xlo_t[:, 2 * cp:2 * cp + 2, :],
                            xlo[:, xbase + cp * 2 * BLK:xbase + (cp + 1) * 2 * BLK])
                    if blk == 0 and weights_after_first_xt:
                        load_weights(chunked=True)

                    # ---- router: three comped passes into one [8,tok] group
                    lg = ps_r.tile([E, BLK], F32, tag="lg")
                    for cp in range(NCP):
                        hi = c8_t[:, 2 * cp:2 * cp + 2, 0:E]
                        lo = c8_t[:, 2 * cp:2 * cp + 2, E:2 * E]
                        s = slice(2 * cp, 2 * cp + 2)
                        nc.tensor.matmul(lg[:], hi, xq_t[:, s, :],
                                         start=(cp == 0), stop=False,
                                         perf_mode=DR)
                        nc.tensor.matmul(lg[:], hi, xlo_t[:, s, :],
                                         start=False, stop=False, perf_mode=DR)
                        nc.tensor.matmul(lg[:], lo, xq_t[:, s, :],
                                         start=False, stop=(cp == NCP - 1),
                                         perf_mode=DR)

                    # exp(l) = (1+tanh(l/2))/(1-tanh(l/2)); tanh stays on the
                    # gelu act table.  l = lg/(SX*SWR) + br.
                    th = work.tile([E, BLK], F32, tag="th")
                    nc.scalar.activation(th[:], lg[0:E, :], AF.Tanh,
                                         scale=0.5 / (SX * SWR), bias=brh_t)
                    num = work.tile([E, BLK], F32, tag="num")
                    nc.vector.tensor_scalar_add(num[:], th[:], 1.0)
                    den = work.tile([E, BLK], F32, tag="den")
                    nc.vector.tensor_scalar(den[:], th[:], -1.0, 1.0,
                                            ALU.mult, ALU.add)
                    rden = work.tile([E, BLK], F32, tag="rden")
                    nc.vector.reciprocal(rden[:], den[:])
                    expT = rt.tile([E, BLK], F32R, tag="expT")
                    nc.vector.tensor_tensor(expT[:], num[:], rden[:], ALU.mult)

                    # per-token 1/(SWU*sum_e w): K=8 matmul with SWU column
                    rinv = rt.tile([128, NSUB], F32, tag="rinv")
                    for sub in range(NSUB):
                        sm = ps_r.tile([128, 2], F32, tag="lg")
                        nc.tensor.matmul(
                            sm[:], expT[:, sub * 128:(sub + 1) * 128],
                            ones_t, start=True, stop=True)
                        nc.vector.reciprocal(rinv[:, sub:sub + 1], sm[:, 0:1])

                    # ---- experts: down (main + x-comp + wd-comp) ----
                    sc_pair = [scp.tile([128, 2, BLK], F8, tag=f"sc{pi}",
                                        name=f"sc{pi}")
                               for pi in range(4)]
                    for e in range(E):
                        dn = ps_d.tile([128, BLK], F32, tag="dn")
                        st = wd_t[:, e * NCH:(e + 1) * NCH, :]
                        stlo = wdlo_t[:, e * NCH:(e + 1) * NCH, :]
                        for cp in range(NCP):
                            s = slice(2 * cp, 2 * cp + 2)
                            nc.tensor.matmul(dn[:], st[:, s, :], xq_t[:, s, :],
                                             start=(cp == 0), stop=False,
                                             perf_mode=DR)
                            nc.tensor.matmul(dn[:], st[:, s, :], xlo_t[:, s, :],
                                             start=False, stop=False,
                                             perf_mode=DR)
                            nc.tensor.matmul(dn[:], stlo[:, s, :], xq_t[:, s, :],
                                             start=False, stop=(cp == NCP - 1),
                                             perf_mode=DR)
                        act = actp.tile([128, BLK], F32, tag="act")
                        nc.scalar.activation(act[:], dn[:], AF.Gelu,
                                             scale=1.0 / (SX * SWD),
                                             bias=bdt_t[:, e:e + 1])
                        # w_e broadcast to 128 partitions via ones-row matmul
                        pbc = ps_r.tile([128, BLK], F32, tag="lg")
                        nc.tensor.matmul(pbc[:], sel_t[e], expT[:],
                                         start=True, stop=True)
                        nc.vector.tensor_tensor(
                            sc_pair[e // 2][:, e % 2, :], act[:], pbc[:],
                            ALU.mult)

                    # ---- up: all experts + bias into one PSUM group ----
                    for sub in range(NSUB):
                        ssub = slice(sub * 128, (sub + 1) * 128)
                        ot = outp.tile([128, D], BF16, tag="ot")
                        for dc in range(NDC):
                            up = ps_u.tile([128, BLK], F32, tag="up")
                            nc.tensor.matmul(
                                up[:], expT[:, ssub],
                                bu_t[:, dc * BLK:(dc + 1) * BLK],
                                start=True, stop=False)
                            for pi in range(4):
                                w_i = (pi * NDC + dc) * 2
                                nc.tensor.matmul(
                                    up[:], sc_pair[pi][:, :, ssub],
                                    wu_t[:, w_i:w_i + 2, :],
                                    start=False, stop=False, perf_mode=DR)
                                nc.tensor.matmul(
                                    up[:], sc_pair[pi][:, :, ssub],
                                    wulo_t[:, w_i:w_i + 2, :],
                                    start=False, stop=(pi == 3), perf_mode=DR)
                            nc.scalar.activation(
                                ot[:, dc * BLK:(dc + 1) * BLK], up[:], AF.Copy,
                                scale=rinv[:, sub:sub + 1])
                        nc.sync.dma_start(
                            out[blk * BLK + sub * 128:blk * BLK + (sub + 1) * 128, :],
                            ot[:])

            if loop_n == 1:
                emit_body(weights_after_first_xt=True)
            else:
                load_weights()
                with tc.For_i(0, loop_n, 1):
                    emit_body()
    nc.compile()
    return nc


def prep_inputs(x, Wr, br, Wd, bd, Wu, bu):
    """Host-side quantization + packing: per-core xq/xlo + shared weights."""
    x = np.asarray(x, dtype=np.float32)
    Wr = np.asarray(Wr, dtype=np.float32)
    br_ = np.asarray(br, dtype=np.float32)
    Wd = np.asarray(Wd, dtype=np.float32)
    bd_ = np.asarray(bd, dtype=np.float32)
    Wu = np.asarray(Wu, dtype=np.float32)
    bu_ = np.asarray(bu, dtype=np.float32)

    # wd: [128(p), e, c, h] with d = c*128+p
    wd8, wdlo8 = _q8(Wd.reshape(E, NCH, 128, H).transpose(2, 0, 1, 3)
                     .reshape(128, E * NCH * H), SWD)
    # wu: [128(h), pi, dc, i, n] with expert 2*pi+i, d = dc*512+n
    wu_p = (Wu.transpose(1, 0, 2)                 # [H, E, D]
            .reshape(128, 4, 2, NDC, BLK)         # (h, pi, i, dc, n)
            .transpose(0, 1, 3, 2, 4)             # (h, pi, dc, i, n)
            .reshape(128, 2 * E * BLK))
    wu8, wulo8 = _q8(wu_p, SWU)

    # router stationary: [128(p), (cp,i)=c, m] m<8: wr_hi, m>=8: wr_lo
    wr_s = Wr.reshape(NCH, 128, E).transpose(1, 0, 2)     # [p, c, e]
    wr8, wrlo8 = _q8(wr_s, SWR)
    c8p = np.zeros((128, NCH, 16), dtype=E4M3)
    c8p[:, :, 0:E] = wr8
    c8p[:, :, E:16] = wrlo8
    c8p = c8p.reshape(128, NCH * 16)

    cfp = np.zeros((128, CF_W), dtype=np.float32)
    cfp[0:E, CF_BU:CF_BU + D] = _to_f32r(bu_ * SWU)
    for e in range(E):
        cfp[e, CF_SEL + e * 128:CF_SEL + (e + 1) * 128] = 1.0
    cfp[0:E, CF_BRH] = 0.5 * br_
    cfp[0:E, CF_ONES:CF_ONES + 2] = SWU
    cfp[:, CF_BDT:CF_BDT + 8] = bd_.T

    shared = dict(wd=wd8, wdlo=wdlo8, wu=wu8, wulo=wulo8, c8=c8p, cf=cfp)
    in_maps = []
    for core in range(N_CORES):
        xp = (x[core].reshape(NBLK, BLK, NCH, 128)
              .transpose(3, 0, 2, 1).reshape(128, NBLK * NCH * BLK))
        xq8, xlo8 = _q8(xp, SX)
        in_maps.append(dict(shared, xq=xq8, xlo=xlo8))
    return in_maps


_NC_CACHE = {}


def get_nc(loop_n=1):
    if loop_n not in _NC_CACHE:
        _NC_CACHE[loop_n] = build_nc(loop_n)
    return _NC_CACHE[loop_n]


def kernel(x, Wr, br, Wd, bd, Wu, bu):
    nc = get_nc()
    in_maps = prep_inputs(x, Wr, br, Wd, bd, Wu, bu)
    res = run_bass_kernel_spmd(nc, in_maps, list(range(N_CORES)))
    out = np.stack([np.asarray(res.results[i]["out"]).astype(np.float32)
                    for i in range(N_CORES)], axis=0)
    return out
